# revision 2
# baseline (speedup 1.0000x reference)
"""Trainium2 Bass kernel v2 for nn_LookupTableMy (embedding gathers + LSTM + window dots).

Changes vs baseline:
  - All gathers from an f16 copy of the table (half DMA bytes, no e2 casts,
    f16 transposes at 1cyc/row, f16 xT feeds f16 x-part matmuls).
  - i/f/o gates' W_hh@h computed with fp8e4 DoubleRow matmuls (2 K-blocks per
    pass): per-chunk h-part drops 1536cyc -> 512cyc. g gate stays f16
    (it feeds c directly; fp8 there breaks the 2e-2 gate). Max-rel-err
    simulated offline: 1.70e-2.
  - h kept as hT16 [128dims, 3, 512batch] f16; fp8 pair tiles hpA=(h0,h1),
    hpB=(h2, zero-weight block) cast once per step on DVE.
  - Transposes pack 4 chunks into one [128,4,128] f16 PSUM tile -> one DVE
    copy per wave.
  - e2 gathered chunk-major so the tail (per-cb window dots) pipelines
    against the trailing gathers.
  - Per-chunk ACT with per-partition bias (short recurrence chain).

Sharding: data-parallel over batch: 4096 samples -> 8 cores x 512.
"""

import sys
from contextlib import ExitStack

for _p in ("/opt/trn_rl_repo",):
    if _p not in sys.path:
        sys.path.insert(0, _p)

import numpy as np
import ml_dtypes

import concourse.bass as bass
import concourse.tile as tile
import concourse.bacc as bacc
import concourse.mybir as mybir
from concourse import bass_utils
from concourse.bass import IndirectOffsetOnAxis
from concourse.masks import make_identity

F32 = mybir.dt.float32
F16 = mybir.dt.float16
FP8 = mybir.dt.float8e4
I32 = mybir.dt.int32
AF = mybir.ActivationFunctionType
ALU = mybir.AluOpType
AX = mybir.AxisListType
DR = mybir.MatmulPerfMode.DoubleRow
NP_FP8 = ml_dtypes.float8_e4m3  # TRN flavor (max 240, has inf)

V, D, OUT = 100000, 128, 2
H = 3 * D
B, L1, L2 = 4096, 19, 20
NWIN = 18
NCORES = 8
BC = B // NCORES          # 512 samples per core
NB = BC // 128            # 4 batch chunks of 128
NJ = 3                    # hidden segments of 128
# gate-chunk index: gc = q*3 + j, q in (i,f,g,o). ifo chunks in order:
IFO_GCS = [0, 1, 2, 3, 4, 5, 9, 10, 11]
CI_OF_GC = {gc: ci for ci, gc in enumerate(IFO_GCS)}

_cache = {}


def _emit_step(nc, t, xts, hT_prev, hpA_prev, hpB_prev, c_sb, pools):
    """Emit one LSTM step (per-chunk j-major; the empirically fastest order).

    h is produced twice: straight to fp8 pair tiles (hpA=(h0,h1), hpB=(h2,-))
    feeding the next step's DoubleRow matmuls, and as f16 for the g-gate.
    """
    psum_z, gates, tcp, igp, hc = pools
    gate_t = {}
    for j in range(NJ):
        for q, func in ((0, AF.Sigmoid), (1, AF.Sigmoid), (2, AF.Tanh),
                        (3, AF.Sigmoid)):
            gc = q * NJ + j
            ps = psum_z.tile([128, 512], F32, tag="z", name=f"z{t}_{gc}")
            nc.tensor.matmul(
                out=ps[:], lhsT=nc.wx16[:, gc * 128:(gc + 1) * 128],
                rhs=xts[t][:], start=True, stop=False)
            if q != 2:
                ci = CI_OF_GC[gc]
                nc.tensor.matmul(
                    out=ps[:], lhsT=nc.whA8[:, :, ci * 128:(ci + 1) * 128],
                    rhs=hpA_prev[:], start=False, stop=False, perf_mode=DR)
                nc.tensor.matmul(
                    out=ps[:], lhsT=nc.whB8[:, :, ci * 128:(ci + 1) * 128],
                    rhs=hpB_prev[:], start=False, stop=True, perf_mode=DR)
            else:
                for kj in range(NJ):
                    nc.tensor.matmul(
                        out=ps[:], lhsT=nc.whG16[:, kj, j * 128:(j + 1) * 128],
                        rhs=hT_prev[:, kj, :], start=False,
                        stop=(kj == NJ - 1))
            gsb = gates.tile([128, 512], F16, tag=f"g{q}", name=f"g{t}_{gc}")
            nc.scalar.activation(
                out=gsb[:], in_=ps[:], func=func,
                bias=nc.bias_sb[:, gc:gc + 1], scale=1.0)
            gate_t[(q, j)] = gsb

    # c update + tanh(c) per segment; c in-place [128, 3, 512] f16
    tc_t = [None] * NJ
    for j in range(NJ):
        cj = c_sb[:, j, :]
        if t == 0:
            nc.vector.tensor_tensor(out=cj, in0=gate_t[(0, j)][:],
                                    in1=gate_t[(2, j)][:], op=ALU.mult)
        else:
            ig = igp.tile([128, 512], F16, tag="ig", name=f"ig{t}_{j}")
            nc.vector.tensor_tensor(out=ig[:], in0=gate_t[(0, j)][:],
                                    in1=gate_t[(2, j)][:], op=ALU.mult)
            nc.vector.tensor_tensor(out=cj, in0=gate_t[(1, j)][:], in1=cj,
                                    op=ALU.mult)
            nc.vector.tensor_tensor(out=cj, in0=cj, in1=ig[:], op=ALU.add)
        tc = tcp.tile([128, 512], F16, tag="tc", name=f"tc{t}_{j}")
        nc.scalar.activation(out=tc[:], in_=cj, func=AF.Tanh)
        tc_t[j] = tc

    hT_t = hc.tile([128, NJ, 512], F16, tag="hT", name=f"hT{t}")
    for j in range(NJ):
        nc.vector.tensor_tensor(out=hT_t[:, j, :], in0=gate_t[(3, j)][:],
                                in1=tc_t[j][:], op=ALU.mult)
    hpA_t = hc.tile([128, 2, 512], FP8, tag="hpA", name=f"hpA{t}")
    nc.vector.tensor_copy(out=hpA_t[:], in_=hT_t[:, 0:2, :])
    hpB_t = hc.tile([128, 2, 512], FP8, tag="hpB", name=f"hpB{t}")
    nc.vector.tensor_copy(out=hpB_t[:, 0, :], in_=hT_t[:, 2, :])
    return hT_t, hpA_t, hpB_t


def _build():
    if "nc" in _cache:
        return _cache["nc"]

    nc = bacc.Bacc(
        "TRN2",
        target_bir_lowering=False,
        debug=False,
        enable_asserts=False,
        num_devices=NCORES,
    )

    emb16_d = nc.dram_tensor("emb16", [V, D], F16, kind="ExternalInput").ap()
    wx_d = nc.dram_tensor("wx16", [128, 12 * 128], F16, kind="ExternalInput").ap()
    whA_d = nc.dram_tensor("whA8", [128, 2, 9 * 128], FP8, kind="ExternalInput").ap()
    whB_d = nc.dram_tensor("whB8", [128, 2, 9 * 128], FP8, kind="ExternalInput").ap()
    whG_d = nc.dram_tensor("whG16", [128, NJ, 3 * 128], F16, kind="ExternalInput").ap()
    bias_d = nc.dram_tensor("bias", [12, 128], F32, kind="ExternalInput").ap()
    lwb_d = nc.dram_tensor("lwb", [1, 4], F32, kind="ExternalInput").ap()
    idx1_d = nc.dram_tensor("idx1", [128, NB, L1], I32, kind="ExternalInput").ap()
    idx2_d = nc.dram_tensor("idx2", [128, NB, L2], I32, kind="ExternalInput").ap()
    out_d = nc.dram_tensor("out", [BC, OUT], F32, kind="ExternalOutput").ap()

    with tile.TileContext(nc) as tc, ExitStack() as ctx:
        singles = ctx.enter_context(tc.tile_pool(name="singles", bufs=1))
        xtp = ctx.enter_context(tc.tile_pool(name="xtp", bufs=4))
        psum_z = ctx.enter_context(tc.tile_pool(name="psum_z", bufs=6, space="PSUM"))
        psum_tr = ctx.enter_context(tc.tile_pool(name="psum_tr", bufs=2, space="PSUM"))
        gates = ctx.enter_context(tc.tile_pool(name="gates", bufs=3))
        hc = ctx.enter_context(tc.tile_pool(name="hc", bufs=2))
        tcp = ctx.enter_context(tc.tile_pool(name="tcp", bufs=3))
        igp = ctx.enter_context(tc.tile_pool(name="igp", bufs=2))
        prodp = ctx.enter_context(tc.tile_pool(name="prodp", bufs=2))
        small = ctx.enter_context(tc.tile_pool(name="small", bufs=2))

        # ---- constants (idx first: gathers wait on them) ----
        idx1_sb = singles.tile([128, NB, L1], I32, tag="idx1")
        nc.sync.dma_start(out=idx1_sb[:], in_=idx1_d)
        idx2_sb = singles.tile([128, NB, L2], I32, tag="idx2")
        nc.sync.dma_start(out=idx2_sb[:], in_=idx2_d)
        nc.wx16 = singles.tile([128, 12 * 128], F16, tag="wx16")
        nc.sync.dma_start(out=nc.wx16[:], in_=wx_d)
        nc.whA8 = singles.tile([128, 2, 9 * 128], FP8, tag="whA8")
        nc.sync.dma_start(out=nc.whA8[:], in_=whA_d)
        nc.whB8 = singles.tile([128, 2, 9 * 128], FP8, tag="whB8")
        nc.sync.dma_start(out=nc.whB8[:], in_=whB_d)
        nc.whG16 = singles.tile([128, NJ, 3 * 128], F16, tag="whG16")
        nc.sync.dma_start(out=nc.whG16[:], in_=whG_d)
        nc.bias_sb = singles.tile([128, 12], F32, tag="bias")
        nc.sync.dma_start(out=nc.bias_sb[:], in_=bias_d.rearrange("g p -> p g"))
        lwb_sb = singles.tile([128, 4], F32, tag="lwb")
        nc.gpsimd.dma_start(out=lwb_sb[:], in_=lwb_d.to_broadcast([128, 4]))

        ident16 = singles.tile([128, 128], F16, tag="ident16")
        make_identity(nc, ident16[:])

        # zero-init tiles for t=0 (h(-1) == 0); hpB block1 must be non-NaN
        hT_prev = hc.tile([128, NJ, 512], F16, tag="hT", name="hT_init")
        nc.vector.memset(hT_prev[:], 0.0)
        hpA_prev = hc.tile([128, 2, 512], FP8, tag="hpA", name="hpA_init")
        nc.vector.memset(hpA_prev[:], 0.0)
        hpB_prev = hc.tile([128, 2, 512], FP8, tag="hpB", name="hpB_init")
        nc.vector.memset(hpB_prev[:], 0.0)
        # second pool slot: block 1 must be a real number forever (the DR
        # zero-weight block multiplies it; 0 * garbage-NaN = NaN). Slots
        # rotate, so zeroing both once covers every step.
        hpB_init2 = hc.tile([128, 2, 512], FP8, tag="hpB", name="hpB_init2")
        nc.vector.memset(hpB_init2[:], 0.0)
        c_sb = singles.tile([128, NJ, 512], F16, tag="c_sb")

        # ---- gathers, all on gpsimd (rate-limiting: ~1.41us/instruction).
        # e1 wave-major; e2 (chunk-major) interleaved 3-per-wave into the
        # slack so early chunks' window dots can run under the gather wall.
        g1all = singles.tile([128, L1, NB, 128], F16, tag="g1all")
        g2b = []
        for cb in range(NB):
            t2 = singles.tile([128, L2, 128], F16, tag=f"g2b{cb}",
                              name=f"g2b{cb}")
            g2b.append(t2)

        def _e2_gather(cb, k):
            nc.gpsimd.indirect_dma_start(
                out=g2b[cb][:, k, :],
                out_offset=None,
                in_=emb16_d,
                in_offset=IndirectOffsetOnAxis(
                    ap=idx2_sb[:, cb, k:k + 1], axis=0),
            )

        for w in range(L1):
            for cb in range(NB):
                nc.gpsimd.indirect_dma_start(
                    out=g1all[:, w, cb, :],
                    out_offset=None,
                    in_=emb16_d,
                    in_offset=IndirectOffsetOnAxis(
                        ap=idx1_sb[:, cb, w:w + 1], axis=0),
                )
        for cb in range(NB):
            for k in range(L2):
                _e2_gather(cb, k)

        # ---- waves: transpose e1 (4 chunks -> one psum tile -> one copy) ----
        xts = [None] * L1
        pools = (psum_z, gates, tcp, igp, hc)
        LAG = 2
        for wv in range(L1 + LAG):
            if wv < L1:
                ps = psum_tr.tile([128, NB, 128], F16, tag="trps",
                                  name=f"tp{wv}")
                for cb in range(NB):
                    nc.tensor.transpose(
                        out=ps[:, cb, :],
                        in_=g1all[:, wv, cb, :],
                        identity=ident16[:],
                    )
                xt = xtp.tile([128, NB, 128], F16, tag="xT", name=f"xT{wv}")
                nc.vector.tensor_copy(out=xt[:], in_=ps[:])
                xts[wv] = xt  # 3D AP; matmul flattens contiguous free dims
            t = wv - LAG
            if 0 <= t < L1:
                hT_prev, hpA_prev, hpB_prev = _emit_step(
                    nc, t, xts, hT_prev, hpA_prev, hpB_prev, c_sb, pools)

        # ---- tail per chunk, cb-major: per-(cb,j) mul + tree + reduce.
        # gpsimd (idle once gathers end) takes two early chains; DVE the rest.
        a_t, negm_t, se_t, lse_t = [], [], [], []
        for cb in range(NB):
            ps = psum_tr.tile([128, NB, 128], F16, tag="trps", name=f"htp{cb}")
            for j in range(NJ):
                nc.tensor.transpose(
                    out=ps[:, j, :],
                    in_=hT_prev[:, j, cb * 128:(cb + 1) * 128],
                    identity=ident16[:],
                )
            hb = singles.tile([128, NJ, 128], F16, tag=f"hbm{cb}",
                              name=f"hbm{cb}")
            nc.vector.tensor_copy(out=hb[:], in_=ps[:, 0:NJ, :])

            s = small.tile([128, NJ, L2], F32, tag=f"s{cb}", name=f"s{cb}")
            for j in range(NJ):
                hseg = hb[:, j, :]
                hbc = bass.AP(
                    tensor=hseg.tensor,
                    offset=hseg.offset,
                    ap=[hseg.ap[0], [0, L2], hseg.ap[1]],
                )
                prod = prodp.tile([128, L2, D], F16, tag="prod",
                                  name=f"pr{cb}_{j}")
                nc.vector.tensor_tensor(out=prod[:], in0=g2b[cb][:], in1=hbc,
                                  op=ALU.mult)
                h1 = prodp.tile([128, L2, D // 2], F16, tag="half1",
                                name=f"h1_{cb}_{j}")
                nc.vector.tensor_tensor(
                    out=h1[:], in0=prod[:, :, 0:D // 2],
                    in1=prod[:, :, D // 2:D], op=ALU.add)
                h2 = prodp.tile([128, L2, D // 4], F16, tag="half2",
                                name=f"h2_{cb}_{j}")
                nc.vector.tensor_tensor(
                    out=h2[:], in0=h1[:, :, 0:D // 4],
                    in1=h1[:, :, D // 4:D // 2], op=ALU.add)
                nc.vector.tensor_reduce(out=s[:, j, :], in_=h2[:], axis=AX.X,
                                        op=ALU.add)
            rs0 = small.tile([128, NWIN], F32, tag="rs0", name=f"rs0_{cb}")
            nc.vector.tensor_tensor(out=rs0[:], in0=s[:, 0, 0:NWIN],
                                    in1=s[:, 1, 1:NWIN + 1], op=ALU.add)
            rs1 = small.tile([128, NWIN], F32, tag="rs1", name=f"rs1_{cb}")
            nc.vector.tensor_tensor(out=rs1[:], in0=rs0[:],
                                    in1=s[:, 2, 2:NWIN + 2], op=ALU.add)
            ms = small.tile([128, 1], F32, tag="ms", name=f"ms{cb}")
            nc.vector.tensor_reduce(out=ms[:], in_=rs1[:], axis=AX.X,
                                    op=ALU.max)
            a = small.tile([128, OUT], F32, tag=f"a{cb}", name=f"a{cb}")
            nc.vector.scalar_tensor_tensor(
                out=a[:], in0=lwb_sb[:, 0:2], scalar=ms[:, 0:1],
                in1=lwb_sb[:, 2:4], op0=ALU.mult, op1=ALU.add)
            negm = small.tile([128, 1], F32, tag=f"negm{cb}", name=f"negm{cb}")
            nc.vector.tensor_reduce(out=negm[:], in_=a[:], axis=AX.X,
                                    op=ALU.max, negate=True)
            a_t.append(a)
            negm_t.append(negm)
        for cb in range(NB):
            e = small.tile([128, OUT], F32, tag=f"e{cb}", name=f"e{cb}")
            se = small.tile([128, 1], F32, tag=f"se{cb}", name=f"se{cb}")
            nc.scalar.activation(out=e[:], in_=a_t[cb][:], func=AF.Exp,
                                 bias=negm_t[cb][:, 0:1], accum_out=se[:])
            se_t.append(se)
        for cb in range(NB):
            lse = small.tile([128, 1], F32, tag=f"lse{cb}", name=f"lse{cb}")
            nc.scalar.activation(out=lse[:], in_=se_t[cb][:], func=AF.Ln)
            lse_t.append(lse)
        for cb in range(NB):
            combo = small.tile([128, 1], F32, tag=f"combo{cb}",
                               name=f"combo{cb}")
            nc.vector.tensor_tensor(out=combo[:], in0=negm_t[cb][:],
                                    in1=lse_t[cb][:], op=ALU.subtract)
            ot = small.tile([128, OUT], F32, tag=f"ot{cb}", name=f"ot{cb}")
            nc.vector.tensor_scalar_add(ot[:], a_t[cb][:], combo[:, 0:1])
            nc.sync.dma_start(out=out_d[cb * 128:(cb + 1) * 128, :], in_=ot[:])

    nc.compile()
    _cache["nc"] = nc
    return nc


def _prep_weights(W_ih, W_hh, b_ih, b_hh):
    wfullT = np.concatenate([W_ih, W_hh], axis=1).T.astype(np.float32)  # [512, 1536]
    k = [np.ascontiguousarray(wfullT[i * 128:(i + 1) * 128]) for i in range(4)]
    wx16 = k[0].astype(np.float16)                                      # [128, 1536]
    ifo_cols = np.concatenate(
        [np.arange(gc * 128, (gc + 1) * 128) for gc in IFO_GCS])        # [1152]
    whA8 = np.stack([k[1][:, ifo_cols], k[2][:, ifo_cols]], axis=1)     # [128,2,1152]
    whB8 = np.stack([k[3][:, ifo_cols], np.zeros_like(k[3][:, ifo_cols])], axis=1)
    whA8 = whA8.astype(NP_FP8)
    whB8 = whB8.astype(NP_FP8)
    g_cols = np.arange(6 * 128, 9 * 128)
    whG16 = np.stack([k[1][:, g_cols], k[2][:, g_cols], k[3][:, g_cols]],
                     axis=1).astype(np.float16)                         # [128,3,384]
    bias = np.ascontiguousarray(
        (b_ih + b_hh).astype(np.float32).reshape(12, 128))
    return (np.ascontiguousarray(wx16), np.ascontiguousarray(whA8),
            np.ascontiguousarray(whB8), np.ascontiguousarray(whG16), bias)


def kernel(input1, input2, emb, W_ih, W_hh, b_ih, b_hh, lin_w, lin_b,
           _trace=False):
    input1 = np.ascontiguousarray(np.asarray(input1, dtype=np.int64).astype(np.int32))
    input2 = np.ascontiguousarray(np.asarray(input2, dtype=np.int64).astype(np.int32))
    emb16 = np.ascontiguousarray(np.asarray(emb, dtype=np.float32).astype(np.float16))
    wx16, whA8, whB8, whG16, bias = _prep_weights(
        np.asarray(W_ih, dtype=np.float32), np.asarray(W_hh, dtype=np.float32),
        np.asarray(b_ih, dtype=np.float32), np.asarray(b_hh, dtype=np.float32))
    lin_w = np.asarray(lin_w, dtype=np.float32)
    lin_b = np.asarray(lin_b, dtype=np.float32)
    lwb = np.ascontiguousarray(
        np.array([[lin_w[0, 0], lin_w[1, 0], lin_b[0], lin_b[1]]],
                 dtype=np.float32))

    nc = _build()

    in_maps = []
    for c in range(NCORES):
        i1 = input1[c * BC:(c + 1) * BC].reshape(NB, 128, L1).transpose(1, 0, 2)
        i2 = input2[c * BC:(c + 1) * BC].reshape(NB, 128, L2).transpose(1, 0, 2)
        in_maps.append({
            "emb16": emb16, "wx16": wx16, "whA8": whA8, "whB8": whB8,
            "whG16": whG16, "bias": bias, "lwb": lwb,
            "idx1": np.ascontiguousarray(i1), "idx2": np.ascontiguousarray(i2),
        })

    res = bass_utils.run_bass_kernel_spmd(
        nc, in_maps, core_ids=list(range(NCORES)), trace=_trace)
    if _trace:
        kernel.last_results = res
    out = np.concatenate([res.results[c]["out"] for c in range(NCORES)], axis=0)
    return out


if __name__ == "__main__":
    rng = np.random.default_rng(0)
    inputs = {
        "input1": rng.integers(0, V, (B, L1), dtype=np.int32),
        "input2": rng.integers(0, V, (B, L2), dtype=np.int32),
        "emb": rng.standard_normal((V, D), dtype=np.float32),
        "W_ih": (rng.standard_normal((4 * H, D), dtype=np.float32) * 0.05),
        "W_hh": (rng.standard_normal((4 * H, H), dtype=np.float32) * 0.05),
        "b_ih": (rng.standard_normal(4 * H).astype(np.float32) * 0.05),
        "b_hh": (rng.standard_normal(4 * H).astype(np.float32) * 0.05),
        "lin_w": rng.standard_normal((OUT, 1), dtype=np.float32),
        "lin_b": rng.standard_normal(OUT).astype(np.float32),
    }
    out = kernel(**inputs)
    print(out.shape, out[:2])
